# revision 16
# baseline (speedup 1.0000x reference)
"""Trainium2 Bass kernel for nn_DecoderRNN (show-attend-tell image captioning decoder).

Data-parallel over batch across 8 NeuronCores (strided by sorted caption
length for load balance); the ragged sequential scan runs locally per core.
Per step: Bahdanau attention (tanh/softmax), LSTM cell; the big
[B*T,H]@[H,V] output projection is deferred to one batched matmul at the end.

Tricks:
- ragged skip: only the active batch slots are processed each step (schedule
  baked at build time from the actual `lengths` input; identical across cores
  because slots are dealt round-robin from the length-sorted batch).
- everything bf16 on the matmul paths, fp32 PSUM accumulation, fp32 cell state.
- sigmoid is rewritten as tanh (sigma(x) = (1+tanh(x/2))/2) with the cell and
  hidden state rescaled by 2 (host halves Whh/w2/fcW to compensate), so every
  scalar-engine op per step (tanh/exp) lives in one activation table -> no
  per-step 1.3us table reloads.
- masked LSTM updates via copy_predicated; masked rows of the stored hidden
  states make the final fc write exact zeros for padded/finished positions.
"""

import os
import sys

import numpy as np

for _p in ("/opt/trn_rl_repo",):
    if _p not in sys.path and os.path.isdir(_p):
        sys.path.insert(0, _p)

import ml_dtypes

import concourse.bass as bass
import concourse.bacc as bacc
import concourse.tile as tile
import concourse.mybir as mybir
from concourse import bass_utils
from concourse.bass import IndirectOffsetOnAxis

BF16 = mybir.dt.bfloat16
F32 = mybir.dt.float32
U32 = mybir.dt.uint32
U8 = mybir.dt.uint8
AF = mybir.ActivationFunctionType
ALU = mybir.AluOpType

NCORES = 8
B, P, E, H, V, L = 128, 196, 512, 512, 10000, 32
T = L - 1          # 31 decode steps
TP = L             # padded time dim in storage (slot-major [slot, 32] grid)
S = B // NCORES    # 16 batch slots per core
EC = E // 128      # 4 chunks of the embed dim
HC = H // 128
GC = (4 * H) // 128  # 16 gate-row chunks (order i,f,o,g after host permute)
P0N = 128
P1N = P - 128      # 68
BT = S * T         # 496 (slot,t) entries, t-major flat index = t*S + s
BTP = 512          # padded for the gather tiles

_f32 = np.float32
_bf16 = ml_dtypes.bfloat16


def _build_program(m_sched, has_fcb):
    """Build the SPMD Bass program (identical on all cores; per-core data
    differs only through input tensors). m_sched[t] = #slots live at step t."""
    nc = bacc.Bacc(
        "TRN2",
        target_bir_lowering=False,
        debug=False,
        enable_asserts=False,
        num_devices=NCORES,
    )

    d = {}
    def inp(name, shape, dt):
        d[name] = nc.dram_tensor(name, shape, dt, kind="ExternalInput")
    inp("imT", [E, S, P], BF16)
    inp("imgs", [P, S, E], BF16)
    inp("w1T", [E, E], BF16)
    inp("w2T", [H, E], BF16)
    inp("b12", [1, E], BF16)
    inp("vv", [E, 1], BF16)
    inp("WcT", [E, 4 * H], BF16)
    inp("WhT", [H, 4 * H], BF16)
    inp("WeT", [E, 4 * H], BF16)
    inp("bg", [1, 4 * H], BF16)
    inp("embW", [V, E], BF16)
    inp("offs", [4, 128], U32)
    inp("maskf", [1, BT], F32)
    inp("maskP", [S, T], F32)
    inp("fcWT", [H, V], BF16)
    inp("fcb", [1, V], BF16)
    inp("maskRow", [1, S * TP], BF16)
    inp("eye", [128, 128], BF16)
    d["preds"] = nc.dram_tensor("preds", [S, TP, V], F32, kind="ExternalOutput")
    d["alphas"] = nc.dram_tensor("alphas", [S, TP, P], F32, kind="ExternalOutput")

    with tile.TileContext(nc) as tc:
        _body(nc, tc, m_sched, has_fcb, d)
    nc.compile()
    return nc


def _body(nc, tc, m_sched, has_fcb, d):
    from contextlib import ExitStack

    ctx = ExitStack()
    with ctx:
        pers = ctx.enter_context(tc.tile_pool(name="pers", bufs=1))
        psum1 = ctx.enter_context(
            tc.tile_pool(name="psum1", bufs=1, space=bass.MemorySpace.PSUM)
        )

        # ---------- persistent SBUF ----------
        imgs0 = pers.tile([128, S, E], BF16, tag="imgs0")
        imgs1 = pers.tile([P1N, S, E], BF16, tag="imgs1")
        FP = pers.tile([128, EC, P, S], BF16, tag="FP")      # feat_proj^T
        score = pers.tile([128, EC, P, S], BF16, tag="score")
        xegT = pers.tile([128, GC, BT], BF16, tag="xegT")
        Wc = pers.tile([128, EC, 4 * H], BF16, tag="Wc")
        Wh = pers.tile([128, HC, 4 * H], BF16, tag="Wh")
        w2T = pers.tile([128, HC, E], BF16, tag="w2T")
        v_s = pers.tile([128, EC], BF16, tag="v_s")
        Hall = pers.tile([128, HC, S, TP], BF16, tag="Hall")
        hT = pers.tile([128, HC, S], BF16, tag="hT")         # stores 2*h
        cT = pers.tile([128, HC, S], F32, tag="cT")          # stores 2*c
        hpT = pers.tile([128, EC, S], BF16, tag="hpT")
        mask_u8 = pers.tile([128, T, S], U8, tag="mask_u8")
        maskb_s = pers.tile([128, T, S], BF16, tag="maskb_s")
        maskP_s = pers.tile([S, T], F32, tag="maskP_s")
        ones_s = pers.tile([1, BT], BF16, tag="ones_s")
        eye_s = pers.tile([128, 128], BF16, tag="eye_s")
        ti = pers.tile([128, HC, S], BF16, tag="ti")
        tf_ = pers.tile([128, HC, S], BF16, tag="tf_")
        to = pers.tile([128, HC, S], BF16, tag="to")
        tg = pers.tile([128, HC, S], BF16, tag="tg")
        tcs = pers.tile([128, HC, S], BF16, tag="tcs")
        u1 = pers.tile([128, HC, S], F32, tag="u1")
        u2 = pers.tile([128, HC, S], F32, tag="u2")
        cnew = pers.tile([128, HC, S], F32, tag="cnew")
        hnew = pers.tile([128, HC, S], BF16, tag="hnew")
        gsum = pers.tile([128, GC, S], BF16, tag="gsum")
        xcT = pers.tile([128, EC, S], BF16, tag="xcT")
        alphaT = pers.tile([128, 2, S], BF16, tag="alphaT")
        mx = pers.tile([S, 1], F32, tag="mx")
        ssum = pers.tile([S, 1], F32, tag="ssum")
        rs = pers.tile([S, 1], F32, tag="rs")
        rs2 = pers.tile([S, 1], F32, tag="rs2")
        exp_s = pers.tile([S, P], BF16, tag="exp_s")
        eT_sb = pers.tile([128, 2, S], BF16, tag="eT_sb")
        mask4 = pers.tile([128, HC, S], U8, tag="mask4")
        maskb4 = pers.tile([128, HC, S], BF16, tag="maskb4")

        # persistent PSUM for the scan (each tile takes a whole 2KB bank)
        ps_h = psum1.tile([128, EC, S], F32, tag="ps_h")
        ps_eT = psum1.tile([128, 2, S], F32, tag="ps_eT")
        ps_e2 = psum1.tile([S, 256], BF16, tag="ps_e2")
        ps_aT = psum1.tile([128, 2, S], BF16, tag="ps_aT")
        ps_ctx = psum1.tile([128, EC, S], F32, tag="ps_ctx")
        ps_g = psum1.tile([128, GC, S], F32, tag="ps_g")

        # ---------- small constants ----------
        nc.vector.memset(ones_s[:], 1.0)
        nc.sync.dma_start(eye_s[:], d["eye"][:])

        for kc in range(HC):
            nc.sync.dma_start(w2T[:, kc, :], d["w2T"][kc * 128 : (kc + 1) * 128, :])
        nc.sync.dma_start(v_s[:], d["vv"][:].rearrange("(c p) o -> p (c o)", p=128))
        for kc in range(EC):
            nc.sync.dma_start(Wc[:, kc, :], d["WcT"][kc * 128 : (kc + 1) * 128, :])
        for kc in range(HC):
            nc.sync.dma_start(Wh[:, kc, :], d["WhT"][kc * 128 : (kc + 1) * 128, :])
        nc.sync.dma_start(maskP_s[:], d["maskP"][:])

        # replicate the (t,slot) mask to all 128 partitions via a K=1 matmul
        with tc.tile_pool(name="bc_ps", bufs=1, space=bass.MemorySpace.PSUM) as bc_ps:
            ones_col = pers.tile([1, 128], BF16, tag="ones_col")
            nc.vector.memset(ones_col[:], 1.0)
            maskf_b = pers.tile([1, BT], BF16, tag="maskf_b")
            maskf_s = pers.tile([1, BT], F32, tag="maskf_s")
            nc.sync.dma_start(maskf_s[:], d["maskf"][:])
            nc.vector.tensor_copy(maskf_b[:], maskf_s[:])
            ps_bc = bc_ps.tile([128, BT], F32, tag="ps_bc")
            nc.tensor.matmul(
                ps_bc[:], ones_col[0:1, :], maskf_b[0:1, :], start=True, stop=True
            )
            nc.vector.tensor_copy(
                maskb_s[:], ps_bc[:].rearrange("p (t s) -> p t s", s=S)
            )
            nc.vector.tensor_copy(mask_u8[:], maskb_s[:])

        # ---------- images in ----------
        nc.sync.dma_start(imgs0[:, :, :], d["imgs"][0:P0N, :, :])
        nc.sync.dma_start(imgs1[:, :, :], d["imgs"][P0N:P, :, :])

        # ---------- feat_proj^T = (img @ w1.T + b1 + b2)^T, layout [e,(ech),p,s] ----------
        with (
            tc.tile_pool(name="fp_tmp", bufs=1) as fp_tmp,
            tc.tile_pool(name="fp_ps", bufs=2, space=bass.MemorySpace.PSUM) as fp_ps,
        ):
            imT_s = fp_tmp.tile([128, EC, S, P], BF16, tag="imT_s")
            w1T_s = fp_tmp.tile([128, EC, E], BF16, tag="w1T_s")
            b12_s = fp_tmp.tile([1, E], BF16, tag="b12_s")
            nc.sync.dma_start(b12_s[:], d["b12"][:])
            for kc in range(EC):
                nc.sync.dma_start(w1T_s[:, kc, :], d["w1T"][kc * 128 : (kc + 1) * 128, :])
                nc.sync.dma_start(
                    imT_s[:, kc, :, :], d["imT"][kc * 128 : (kc + 1) * 128, :, :]
                )
            for j in range(EC):
                for q in range(S // 2):
                    pf = fp_ps.tile([128, 2, P], F32, tag="pf")
                    for kc in range(EC):
                        nc.tensor.matmul(
                            pf[:],
                            w1T_s[:, kc, j * 128 : (j + 1) * 128],
                            imT_s[:, kc, 2 * q : 2 * q + 2, :],
                            start=(kc == 0),
                            stop=False,
                        )
                    nc.tensor.matmul(
                        pf[:].rearrange("p b q -> p (b q)"),
                        b12_s[0:1, j * 128 : (j + 1) * 128],
                        ones_s[0:1, : 2 * P],
                        start=False,
                        stop=True,
                    )
                    # write transposed on the free dims: psum [p2,b2,pp] -> FP [p2,pp,b2]
                    nc.vector.tensor_copy(
                        FP[:, j, :, 2 * q : 2 * q + 2],
                        pf[:].rearrange("p b q -> p q b"),
                    )

        # ---------- embedding gather -> embT, then xeg = emb @ WihE.T + bias ----------
        with (
            tc.tile_pool(name="xe_tmp", bufs=1) as xe_tmp,
            tc.tile_pool(name="xe_g", bufs=2) as xe_g,
            tc.tile_pool(name="xe_ps", bufs=2, space=bass.MemorySpace.PSUM) as xe_ps,
        ):
            offs_s = xe_tmp.tile([128, 4], U32, tag="offs_s")
            for g in range(4):
                nc.sync.dma_start(
                    offs_s[:, g : g + 1], d["offs"][g : g + 1, :].rearrange("o p -> p o")
                )
            embT = xe_tmp.tile([128, EC, BTP], BF16, tag="embT")
            for g in range(4):
                embg = xe_g.tile([128, E], BF16, tag="embg")
                nc.gpsimd.indirect_dma_start(
                    embg[:],
                    None,
                    d["embW"][:],
                    IndirectOffsetOnAxis(ap=offs_s[:, g : g + 1], axis=0),
                )
                for ec in range(EC):
                    nc.sync.dma_start_transpose(
                        embT[:, ec, g * 128 : (g + 1) * 128],
                        embg[:, ec * 128 : (ec + 1) * 128],
                    )
            We_s = xe_tmp.tile([128, EC, 4 * H], BF16, tag="We_s")
            bg_s = xe_tmp.tile([1, 4 * H], BF16, tag="bg_s")
            nc.sync.dma_start(bg_s[:], d["bg"][:])
            for kc in range(EC):
                nc.sync.dma_start(We_s[:, kc, :], d["WeT"][kc * 128 : (kc + 1) * 128, :])
            for mc in range(GC):
                px = xe_ps.tile([128, BT], F32, tag="px")
                for kc in range(EC):
                    nc.tensor.matmul(
                        px[:],
                        We_s[:, kc, mc * 128 : (mc + 1) * 128],
                        embT[:, kc, :BT],
                        start=(kc == 0),
                        stop=False,
                    )
                nc.tensor.matmul(
                    px[:],
                    bg_s[0:1, mc * 128 : (mc + 1) * 128],
                    ones_s[0:1, :BT],
                    start=False,
                    stop=True,
                )
                nc.vector.tensor_copy(xegT[:, mc, :], px[:])

        # ---------- init recurrent state ----------
        nc.vector.memset(hT[:], 0.0)
        nc.vector.memset(cT[:], 0.0)
        nc.vector.memset(hpT[:], 0.0)
        nc.vector.memset(Hall[:], 0.0)

        alpha_pool = ctx.enter_context(tc.tile_pool(name="alpha_out", bufs=3))

        # ---------- the sequential scan ----------
        for t in range(T):
            m = int(m_sched[t])
            if m == 0:
                continue

            # hproj^T[e, b] = sum_k (w2/2)[e,k] H[b,k]   (H = 2h; b2 folded into FP)
            if t > 0:
                for j in range(EC):
                    for kc in range(HC):
                        nc.tensor.matmul(
                            ps_h[:, j, :m],
                            w2T[:, kc, j * 128 : (j + 1) * 128],
                            hT[:, kc, :m],
                            start=(kc == 0),
                            stop=(kc == HC - 1),
                        )
                nc.vector.tensor_copy(hpT[:, :, :m], ps_h[:, :, :m])

            # two slot-halves pipeline tanh (ACT) against e-dot (PE)
            halves = [(0, m)] if m <= 4 else [(0, (m + 1) // 2), ((m + 1) // 2, m)]
            for h0, h1 in halves:
                mh = h1 - h0
                # score = tanh(FP + hproj)   [e_lo, ech, p, slot]
                nc.vector.tensor_tensor(
                    score[:, :, :, h0:h1],
                    FP[:, :, :, h0:h1],
                    hpT[:, :, h0:h1]
                    .rearrange("p c (o s) -> p c o s", o=1)
                    .broadcast_to((128, EC, P, mh)),
                    ALU.add,
                )
                nc.scalar.activation(
                    score[:, :, :, h0:h1], score[:, :, :, h0:h1], AF.Tanh
                )
                # e^T[p, b] = sum_e v[e] * score[e, p, b]  (per-slot, base-0 out)
                for b in range(h0, h1):
                    for pc, (p0, pn) in enumerate(((0, P0N), (P0N, P1N))):
                        for j in range(EC):
                            nc.tensor.matmul(
                                ps_eT[:pn, pc, b : b + 1],
                                score[:, j, p0 : p0 + pn, b],
                                v_s[:, j : j + 1],
                                start=(j == 0),
                                stop=(j == EC - 1),
                            )

            # transpose e^T -> e [slot, p] (two p-chunks into one psum row set)
            nc.vector.tensor_copy(eT_sb[0:P0N, 0, :m], ps_eT[0:P0N, 0, :m])
            nc.vector.tensor_copy(eT_sb[0:P1N, 1, :m], ps_eT[0:P1N, 1, :m])
            nc.tensor.transpose(
                ps_e2[:m, 0:P0N], eT_sb[0:P0N, 0, :m], eye_s[0:P0N, 0:P0N]
            )
            nc.tensor.transpose(
                ps_e2[:m, P0N:P], eT_sb[0:P1N, 1, :m], eye_s[0:P1N, 0:P1N]
            )

            # softmax over p (rows = slots)
            nc.vector.reduce_max(
                mx[:m, :], ps_e2[:m, 0:P], axis=mybir.AxisListType.X, negate=True
            )
            nc.scalar.activation(
                exp_s[:m, :], ps_e2[:m, 0:P], AF.Exp,
                bias=mx[:m, :], scale=1.0, accum_out=ssum[:m, :],
            )
            nc.vector.reciprocal(rs[:m, :], ssum[:m, :])
            nc.vector.tensor_mul(rs2[:m, :], rs[:m, :], maskP_s[:m, t : t + 1])
            a_out = alpha_pool.tile([S, P], F32, tag="a_out")
            nc.vector.tensor_scalar_mul(a_out[:m, :], exp_s[:m, :], rs2[:m, :])
            nc.sync.dma_start(d["alphas"][:m, t, :], a_out[:m, :])
            a_bf = alpha_pool.tile([S, P], BF16, tag="a_bf")
            nc.vector.tensor_scalar_mul(a_bf[:m, :], exp_s[:m, :], rs[:m, :])

            # transpose alpha -> [p, slot]
            nc.tensor.transpose(
                ps_aT[:P0N, 0, :m], a_bf[:m, 0:P0N], eye_s[:m, :m]
            )
            nc.tensor.transpose(
                ps_aT[:P1N, 1, :m], a_bf[:m, P0N:P], eye_s[:m, :m]
            )
            nc.vector.tensor_copy(alphaT[0:P0N, 0, :m], ps_aT[0:P0N, 0, :m])
            nc.vector.tensor_copy(alphaT[0:P1N, 1, :m], ps_aT[0:P1N, 1, :m])

            # ctx^T[e, b] = sum_p images[b, p, e] * alpha[b, p]
            for b in range(m):
                for j in range(EC):
                    nc.tensor.matmul(
                        ps_ctx[:, j, b : b + 1],
                        imgs0[:, b, j * 128 : (j + 1) * 128],
                        alphaT[0:P0N, 0, b : b + 1],
                        start=True,
                        stop=False,
                    )
                    nc.tensor.matmul(
                        ps_ctx[:, j, b : b + 1],
                        imgs1[:, b, j * 128 : (j + 1) * 128],
                        alphaT[0:P1N, 1, b : b + 1],
                        start=False,
                        stop=True,
                    )
            nc.vector.tensor_copy(xcT[:, :, :m], ps_ctx[:, :, :m])

            # gates^T = Wc @ ctx + (Wh/2) @ H  (+ xeg slice), order i,f,o,g
            for mc in range(GC):
                for kc in range(EC):
                    nc.tensor.matmul(
                        ps_g[:, mc, :m],
                        Wc[:, kc, mc * 128 : (mc + 1) * 128],
                        xcT[:, kc, :m],
                        start=(kc == 0),
                        stop=False,
                    )
                for kc in range(HC):
                    nc.tensor.matmul(
                        ps_g[:, mc, :m],
                        Wh[:, kc, mc * 128 : (mc + 1) * 128],
                        hT[:, kc, :m],
                        start=False,
                        stop=(kc == HC - 1),
                    )
            nc.vector.tensor_tensor(
                gsum[:, :, :m], ps_g[:, :, :m], xegT[:, :, t * S : t * S + m], ALU.add
            )

            # LSTM pointwise, all-tanh form (C = 2c, H = 2h):
            #   tx = tanh(pre_x/2) for x in i,f,o ; tg = tanh(pre_g)
            #   C' = 0.5*(tf+1)*C + (ti+1)*tg ; tc = tanh(C'/2) ; H' = (to+1)*tc
            nc.scalar.activation(ti[:, :, :m], gsum[:, 0:4, :m], AF.Tanh, scale=0.5)
            nc.scalar.activation(tf_[:, :, :m], gsum[:, 4:8, :m], AF.Tanh, scale=0.5)
            nc.scalar.activation(to[:, :, :m], gsum[:, 8:12, :m], AF.Tanh, scale=0.5)
            nc.scalar.activation(tg[:, :, :m], gsum[:, 12:16, :m], AF.Tanh)
            nc.vector.scalar_tensor_tensor(
                u1[:, :, :m], ti[:, :, :m], 1.0, tg[:, :, :m], ALU.add, ALU.mult
            )
            nc.vector.scalar_tensor_tensor(
                u2[:, :, :m], tf_[:, :, :m], 1.0, cT[:, :, :m], ALU.add, ALU.mult
            )
            nc.vector.scalar_tensor_tensor(
                cnew[:, :, :m], u2[:, :, :m], 0.5, u1[:, :, :m], ALU.mult, ALU.add
            )
            nc.vector.tensor_copy(
                mask4[:, :, :m], mask_u8[:, t : t + 1, :m].broadcast_to((128, HC, m))
            )
            nc.vector.tensor_copy(
                maskb4[:, :, :m], maskb_s[:, t : t + 1, :m].broadcast_to((128, HC, m))
            )
            nc.vector.copy_predicated(cT[:, :, :m], mask4[:, :, :m], cnew[:, :, :m])
            nc.scalar.activation(tcs[:, :, :m], cT[:, :, :m], AF.Tanh, scale=0.5)
            nc.vector.scalar_tensor_tensor(
                hnew[:, :, :m], to[:, :, :m], 1.0, tcs[:, :, :m], ALU.add, ALU.mult
            )
            nc.vector.copy_predicated(hT[:, :, :m], mask4[:, :, :m], hnew[:, :, :m])
            nc.vector.tensor_mul(Hall[:, :, :m, t], hT[:, :, :m], maskb4[:, :, :m])

        # ---------- deferred fc: preds = mask * (H @ (fcW/2).T (+ fcb)) ----------
        VCH = [(i * 512, min(512, V - i * 512)) for i in range((V + 511) // 512)]
        with (
            tc.tile_pool(name="fc_w", bufs=3) as fc_w,
            tc.tile_pool(name="fc_ps", bufs=2, space=bass.MemorySpace.PSUM) as fc_ps,
            tc.tile_pool(name="fc_c", bufs=1) as fc_c,
        ):
            if has_fcb:
                fcb_s = fc_c.tile([1, V], BF16, tag="fcb_s")
                nc.sync.dma_start(fcb_s[:], d["fcb"][:])
                mrow_s = fc_c.tile([1, S * TP], BF16, tag="mrow_s")
                nc.sync.dma_start(mrow_s[:], d["maskRow"][:])
            for v0, vn in VCH:
                fw = fc_w.tile([128, HC, 512], BF16, tag="fw")
                for kc in range(HC):
                    nc.sync.dma_start(
                        fw[:, kc, :vn],
                        d["fcWT"][kc * 128 : (kc + 1) * 128, v0 : v0 + vn],
                    )
                for mc in range(4):
                    pfc = fc_ps.tile([128, 512], F32, tag="pfc")
                    for kc in range(HC):
                        nc.tensor.matmul(
                            pfc[:, :vn],
                            Hall[:, kc, mc * 4 : (mc + 1) * 4, :],
                            fw[:, kc, :vn],
                            start=(kc == 0),
                            stop=(kc == HC - 1) and not has_fcb,
                        )
                    if has_fcb:
                        nc.tensor.matmul(
                            pfc[:, :vn],
                            mrow_s[0:1, mc * 128 : (mc + 1) * 128],
                            fcb_s[0:1, v0 : v0 + vn],
                            start=False,
                            stop=True,
                        )
                    ofc = fc_w.tile([128, 512], F32, tag="ofc")
                    nc.vector.tensor_copy(ofc[:, :vn], pfc[:, :vn])
                    for s in range(4):
                        nc.sync.dma_start(
                            d["preds"][mc * 4 + s, :, v0 : v0 + vn],
                            ofc[s * 32 : (s + 1) * 32, :vn],
                        )


def _prep(inputs):
    images = np.asarray(inputs["images"], _f32)          # [B, P, E]
    captions = np.asarray(inputs["captions"])            # [B, L] int
    lengths = np.asarray(inputs["lengths"])              # [B] int
    embed_W = np.asarray(inputs["embed_W"], _f32)        # [V, E]
    w1 = np.asarray(inputs["w1"], _f32)                  # [E, E]
    b1 = np.asarray(inputs["b1"], _f32)
    w2 = np.asarray(inputs["w2"], _f32)                  # [E, H]
    b2 = np.asarray(inputs["b2"], _f32)
    v = np.asarray(inputs["v"], _f32)                    # [1, E]
    Wih = np.asarray(inputs["Wih"], _f32)                # [4H, 2E]
    bih = np.asarray(inputs["bih"], _f32)
    Whh = np.asarray(inputs["Whh"], _f32)                # [4H, H]
    bhh = np.asarray(inputs["bhh"], _f32)
    fcW = np.asarray(inputs["fcW"], _f32)                # [V, H]
    fcb = np.asarray(inputs["fcb"], _f32)

    dec = np.clip(lengths.astype(np.int64) - 1, 0, T)    # [B]

    order = np.argsort(-dec, kind="stable")
    core_slots = [
        [int(order[s * NCORES + j]) for s in range(S)] for j in range(NCORES)
    ]
    N_t = np.array([(dec > t).sum() for t in range(T)], np.int64)
    m_sched = [int(-(-n // NCORES)) for n in N_t]

    # gate-row permutation: torch order i,f,g,o -> i,f,o,g
    perm = np.concatenate(
        [np.arange(0, H), np.arange(H, 2 * H), np.arange(3 * H, 4 * H),
         np.arange(2 * H, 3 * H)]
    )
    WihE = Wih[:, :E][perm]
    WihC = Wih[:, E:][perm]
    Whh_p = Whh[perm]
    bg = (bih + bhh)[perm]

    def bf(x):
        return np.ascontiguousarray(x.astype(_bf16))

    shared = {
        "w1T": bf(w1.T),
        "w2T": bf(w2.T * 0.5),                 # h = H/2 folded in
        "b12": bf((b1 + b2)[None, :]),
        "vv": bf(v[0][:, None]),
        "WcT": bf(WihC.T),
        "WhT": bf(Whh_p.T * 0.5),
        "WeT": bf(WihE.T),
        "bg": bf(bg[None, :]),
        "embW": bf(embed_W),
        "fcWT": bf(fcW.T * 0.5),
        "fcb": bf(fcb[None, :]),
        "eye": np.eye(128, dtype=_bf16),
    }
    has_fcb = bool(np.any(fcb != 0.0))

    in_maps = []
    for j in range(NCORES):
        sl = core_slots[j]
        img_c = images[sl]                      # [S, P, E]
        dec_c = dec[sl]
        cap_c = np.asarray(captions)[sl]

        flat = np.zeros(BTP, np.uint32)
        for t in range(T):
            flat[t * S : t * S + S] = cap_c[:, t].astype(np.uint32)
        offs = flat.reshape(4, 128)

        tgrid = np.arange(T)[:, None]           # [T, 1]
        on = (tgrid < dec_c[None, :]).astype(_f32)   # [T, S]
        maskf = on.reshape(1, BT)
        maskP = np.ascontiguousarray(on.T)      # [S, T]
        maskRow = np.zeros((S, TP), _f32)
        maskRow[:, :T] = on.T
        maskRow = maskRow.reshape(1, S * TP).astype(_bf16)

        m = dict(shared)
        m["imT"] = bf(np.ascontiguousarray(img_c.transpose(2, 0, 1)))   # [E, S, P]
        m["imgs"] = bf(np.ascontiguousarray(img_c.transpose(1, 0, 2)))  # [P, S, E]
        m["offs"] = offs
        m["maskf"] = maskf
        m["maskP"] = maskP
        m["maskRow"] = maskRow
        in_maps.append(m)

    return in_maps, m_sched, has_fcb, core_slots


last_results = None  # BassKernelResults of the most recent kernel() call


def kernel(**inputs):
    global last_results
    in_maps, m_sched, has_fcb, core_slots = _prep(inputs)
    nc = _build_program(m_sched, has_fcb)
    res = bass_utils.run_bass_kernel_spmd(nc, in_maps, core_ids=list(range(NCORES)))
    last_results = res
    outs = res.results

    predictions = np.zeros((B, L, V), _f32)
    alphas = np.zeros((B, L, P), _f32)
    for j in range(NCORES):
        pr = np.asarray(outs[j]["preds"], _f32)    # [S, TP, V]
        al = np.asarray(outs[j]["alphas"], _f32)   # [S, TP, P]
        for s in range(S):
            b = core_slots[j][s]
            predictions[b] = pr[s]
            alphas[b] = al[s]

    return predictions, inputs["captions"], inputs["lengths"], alphas


# revision 29
# speedup vs baseline: 1.0214x; 1.0214x over previous
"""Trainium2 Bass kernel for nn_DecoderRNN (show-attend-tell image captioning decoder).

Data-parallel over batch across 8 NeuronCores (strided by sorted caption
length for load balance); the ragged sequential scan runs locally per core.
Per step: Bahdanau attention (tanh/softmax), LSTM cell; the big
[B*T,H]@[H,V] output projection is deferred to one batched matmul at the end.

Tricks:
- ragged skip: only the active batch slots are processed each step (schedule
  baked at build time from the actual `lengths` input; identical across cores
  because slots are dealt round-robin from the length-sorted batch).
- everything bf16 on the matmul paths, fp32 PSUM accumulation, fp32 cell state.
- sigmoid is rewritten as tanh (sigma(x) = (1+tanh(x/2))/2) with the cell and
  hidden state rescaled by 2 (host halves Whh/w2/fcW to compensate), so every
  scalar-engine op per step (tanh/exp) lives in one activation table -> no
  per-step 1.3us table reloads.
- masked LSTM updates via copy_predicated; masked rows of the stored hidden
  states make the final fc write exact zeros for padded/finished positions.
"""

import os
import sys

import numpy as np

for _p in ("/opt/trn_rl_repo",):
    if _p not in sys.path and os.path.isdir(_p):
        sys.path.insert(0, _p)

import ml_dtypes

import concourse.bass as bass
import concourse.bacc as bacc
import concourse.tile as tile
import concourse.mybir as mybir
from concourse import bass_utils
from concourse.bass import IndirectOffsetOnAxis

BF16 = mybir.dt.bfloat16
F32 = mybir.dt.float32
U32 = mybir.dt.uint32
U8 = mybir.dt.uint8
AF = mybir.ActivationFunctionType
ALU = mybir.AluOpType

NCORES = 8
B, P, E, H, V, L = 128, 196, 512, 512, 10000, 32
T = L - 1          # 31 decode steps
TP = L             # padded time dim in storage (slot-major [slot, 32] grid)
S = B // NCORES    # 16 batch slots per core
EC = E // 128      # 4 chunks of the embed dim
HC = H // 128
GC = (4 * H) // 128  # 16 gate-row chunks (order i,f,o,g after host permute)
P0N = 128
P1N = P - 128      # 68
BT = S * T         # 496 (slot,t) entries, t-major flat index = t*S + s
BTP = 512          # padded for the gather tiles

_f32 = np.float32
_bf16 = ml_dtypes.bfloat16


def _build_program(m_sched, has_fcb):
    """Build the SPMD Bass program (identical on all cores; per-core data
    differs only through input tensors). m_sched[t] = #slots live at step t."""
    nc = bacc.Bacc(
        "TRN2",
        target_bir_lowering=False,
        debug=False,
        enable_asserts=False,
        num_devices=NCORES,
    )

    d = {}
    def inp(name, shape, dt):
        d[name] = nc.dram_tensor(name, shape, dt, kind="ExternalInput")
    inp("imT", [E, S, P], BF16)
    inp("imgs", [P, S, E], BF16)
    inp("w1T", [E, E], BF16)
    inp("w2T", [H, E], BF16)
    inp("b12", [1, E], BF16)
    inp("vv", [E, 1], BF16)
    inp("WcT", [E, 4 * H], BF16)
    inp("WhT", [H, 4 * H], BF16)
    inp("WeT", [E, 4 * H], BF16)
    inp("bg", [1, 4 * H], BF16)
    inp("embW", [V, E], BF16)
    inp("offs", [4, 128], U32)
    inp("maskf", [1, BT], F32)
    inp("maskP", [S, T], F32)
    inp("fcWT", [H, V], BF16)
    inp("fcb", [1, V], BF16)
    inp("maskRow", [1, S * TP], BF16)
    inp("eye", [128, 128], BF16)
    d["preds"] = nc.dram_tensor("preds", [TP, S, V], F32, kind="ExternalOutput")
    d["alphas"] = nc.dram_tensor("alphas", [S, TP, P], F32, kind="ExternalOutput")

    with tile.TileContext(nc) as tc:
        _body(nc, tc, m_sched, has_fcb, d)
    nc.compile()
    return nc


def _fc_chunk(nc, d, fc_pools, Hall, c, has_fcb):
    """Emit the output projection for t-chunk c (8 steps): out rows are the
    128 = 16 slots x 8 steps grid; masked rows of Hall are zero so padded
    positions come out exactly 0."""
    fc_w, fc_ps, fcb_s, mrow_s = fc_pools
    t0 = c * 8
    VCH = [(i * 512, min(512, V - i * 512)) for i in range((V + 511) // 512)]
    for v0, vn in VCH:
        fw = fc_w.tile([128, HC, 512], BF16, tag="fw")
        for kc in range(HC):
            nc.sync.dma_start(
                fw[:, kc, :vn], d["fcWT"][kc * 128 : (kc + 1) * 128, v0 : v0 + vn]
            )
        pfc = fc_ps.tile([128, 512], F32, tag="pfc")
        for kc in range(HC):
            nc.tensor.matmul(
                pfc[:, :vn],
                Hall[:, kc, t0 : t0 + 8, :],
                fw[:, kc, :vn],
                start=(kc == 0),
                stop=(kc == HC - 1) and not has_fcb,
            )
        if has_fcb:
            # mask row for the (t, slot) grid rows, t-major flat dt*S + s
            nc.tensor.matmul(
                pfc[:, :vn],
                mrow_s[0:1, t0 * S : (t0 + 8) * S],
                fcb_s[0:1, v0 : v0 + vn],
                start=False,
                stop=True,
            )
        ofc = fc_w.tile([128, 512], F32, tag="ofc")
        nc.vector.tensor_copy(ofc[:, :vn], pfc[:, :vn])
        nc.sync.dma_start(
            d["preds"][t0 : t0 + 8, :, v0 : v0 + vn], ofc[:, :vn]
        )


def _body(nc, tc, m_sched, has_fcb, d):
    from contextlib import ExitStack

    ctx = ExitStack()
    with ctx:
        pers = ctx.enter_context(tc.tile_pool(name="pers", bufs=1))
        psum1 = ctx.enter_context(
            tc.tile_pool(name="psum1", bufs=1, space=bass.MemorySpace.PSUM)
        )

        # ---------- persistent SBUF ----------
        imgs0 = pers.tile([128, S, E], BF16, tag="imgs0")
        imgs1 = pers.tile([P1N, S, E], BF16, tag="imgs1")
        FP = pers.tile([128, EC, P, S], BF16, tag="FP")      # feat_proj^T
        spre = pers.tile([128, EC, P, S], BF16, tag="spre")
        score2 = pers.tile([128, EC, S, P], BF16, tag="score2")
        xegT = pers.tile([128, GC, BT], BF16, tag="xegT")
        Wc = pers.tile([128, EC, 4 * H], BF16, tag="Wc")
        Wh = pers.tile([128, HC, 4 * H], BF16, tag="Wh")
        w2T = pers.tile([128, HC, E], BF16, tag="w2T")
        v_s = pers.tile([128, EC], BF16, tag="v_s")
        Hall = pers.tile([128, HC, TP, S], BF16, tag="Hall")
        hT = pers.tile([128, HC, S], BF16, tag="hT")         # stores 2*h
        cT = pers.tile([128, HC, S], F32, tag="cT")          # stores 2*c
        hpT = pers.tile([128, EC, S], BF16, tag="hpT")
        mask4a = pers.tile([128, T, HC, S], U8, tag="mask4a")
        maskb4a = pers.tile([128, T, HC, S], BF16, tag="maskb4a")
        maskP_s = pers.tile([S, T], F32, tag="maskP_s")
        ones_s = pers.tile([1, BT], BF16, tag="ones_s")
        eye_s = pers.tile([128, 128], BF16, tag="eye_s")
        sig = pers.tile([128, 12, S], BF16, tag="sig")
        tg = pers.tile([128, HC, S], BF16, tag="tg")
        tcs = pers.tile([128, HC, S], BF16, tag="tcs")
        u1 = pers.tile([128, HC, S], F32, tag="u1")
        u2 = pers.tile([128, HC, S], F32, tag="u2")
        cnew = pers.tile([128, HC, S], F32, tag="cnew")
        hnew = pers.tile([128, HC, S], BF16, tag="hnew")
        gsum = pers.tile([128, GC, S], BF16, tag="gsum")
        xcT = pers.tile([128, EC, S], BF16, tag="xcT")
        alphaT = pers.tile([128, 2, S], BF16, tag="alphaT")
        mx = pers.tile([S, 1], F32, tag="mx")
        ssum = pers.tile([S, 1], F32, tag="ssum")
        rs = pers.tile([S, 1], F32, tag="rs")
        rs2 = pers.tile([S, 1], F32, tag="rs2")
        exp_s = pers.tile([S, P], BF16, tag="exp_s")
        eT_sb = pers.tile([128, 2, S], BF16, tag="eT_sb")

        # persistent PSUM for the scan (each tile takes a whole 2KB bank)
        ps_h = psum1.tile([128, EC, S], F32, tag="ps_h")
        ps_eT = psum1.tile([128, 2, EC, S], F32, tag="ps_eT")
        ps_tr = psum1.tile([128, 320], BF16, tag="ps_tr")
        ps_ctx = psum1.tile([128, EC, S], F32, tag="ps_ctx")
        ps_g = psum1.tile([128, GC, S], F32, tag="ps_g")

        # ---------- small constants ----------
        nc.vector.memset(ones_s[:], 1.0)
        nc.sync.dma_start(eye_s[:], d["eye"][:])

        for kc in range(HC):
            nc.sync.dma_start(w2T[:, kc, :], d["w2T"][kc * 128 : (kc + 1) * 128, :])
        nc.sync.dma_start(v_s[:], d["vv"][:].rearrange("(c p) o -> p (c o)", p=128))
        for kc in range(EC):
            nc.sync.dma_start(Wc[:, kc, :], d["WcT"][kc * 128 : (kc + 1) * 128, :])
        for kc in range(HC):
            nc.sync.dma_start(Wh[:, kc, :], d["WhT"][kc * 128 : (kc + 1) * 128, :])
        nc.sync.dma_start(maskP_s[:], d["maskP"][:])

        # replicate the (t,slot) mask to all 128 partitions via a K=1 matmul
        with tc.tile_pool(name="bc_ps", bufs=2, space=bass.MemorySpace.PSUM) as bc_ps:
            ones_col = pers.tile([1, 128], BF16, tag="ones_col")
            nc.vector.memset(ones_col[:], 1.0)
            maskf_b = pers.tile([1, BT], BF16, tag="maskf_b")
            maskf_s = pers.tile([1, BT], F32, tag="maskf_s")
            nc.sync.dma_start(maskf_s[:], d["maskf"][:])
            nc.vector.tensor_copy(maskf_b[:], maskf_s[:])
            for r in range(HC):
                ps_bc = bc_ps.tile([128, BT], F32, tag="ps_bc")
                nc.tensor.matmul(
                    ps_bc[:], ones_col[0:1, :], maskf_b[0:1, :], start=True, stop=True
                )
                nc.vector.tensor_copy(
                    maskb4a[:, :, r, :], ps_bc[:].rearrange("p (t s) -> p t s", s=S)
                )
            nc.vector.tensor_copy(mask4a[:], maskb4a[:])

        # ---------- images in ----------
        nc.sync.dma_start(imgs0[:, :, :], d["imgs"][0:P0N, :, :])
        nc.sync.dma_start(imgs1[:, :, :], d["imgs"][P0N:P, :, :])

        # ---------- feat_proj^T = (img @ w1.T + b1 + b2)^T, layout [e,(ech),p,s] ----------
        with (
            tc.tile_pool(name="fp_tmp", bufs=1) as fp_tmp,
            tc.tile_pool(name="fp_ps", bufs=2, space=bass.MemorySpace.PSUM) as fp_ps,
        ):
            imT_s = fp_tmp.tile([128, EC, S, P], BF16, tag="imT_s")
            w1T_s = fp_tmp.tile([128, EC, E], BF16, tag="w1T_s")
            b12_s = fp_tmp.tile([1, E], BF16, tag="b12_s")
            nc.sync.dma_start(b12_s[:], d["b12"][:])
            for kc in range(EC):
                nc.sync.dma_start(w1T_s[:, kc, :], d["w1T"][kc * 128 : (kc + 1) * 128, :])
                nc.sync.dma_start(
                    imT_s[:, kc, :, :], d["imT"][kc * 128 : (kc + 1) * 128, :, :]
                )
            for j in range(EC):
                for q in range(S // 2):
                    pf = fp_ps.tile([128, 2, P], F32, tag="pf")
                    for kc in range(EC):
                        nc.tensor.matmul(
                            pf[:],
                            w1T_s[:, kc, j * 128 : (j + 1) * 128],
                            imT_s[:, kc, 2 * q : 2 * q + 2, :],
                            start=(kc == 0),
                            stop=False,
                        )
                    nc.tensor.matmul(
                        pf[:].rearrange("p b q -> p (b q)"),
                        b12_s[0:1, j * 128 : (j + 1) * 128],
                        ones_s[0:1, : 2 * P],
                        start=False,
                        stop=True,
                    )
                    # write transposed on the free dims: psum [p2,b2,pp] -> FP [p2,pp,b2]
                    nc.vector.tensor_copy(
                        FP[:, j, :, 2 * q : 2 * q + 2],
                        pf[:].rearrange("p b q -> p q b"),
                    )

        # ---------- embedding gather -> embT, then xeg = emb @ WihE.T + bias ----------
        with (
            tc.tile_pool(name="xe_tmp", bufs=1) as xe_tmp,
            tc.tile_pool(name="xe_g", bufs=2) as xe_g,
            tc.tile_pool(name="xe_ps", bufs=2, space=bass.MemorySpace.PSUM) as xe_ps,
        ):
            offs_s = xe_tmp.tile([128, 4], U32, tag="offs_s")
            for g in range(4):
                nc.sync.dma_start(
                    offs_s[:, g : g + 1], d["offs"][g : g + 1, :].rearrange("o p -> p o")
                )
            embT = xe_tmp.tile([128, EC, BTP], BF16, tag="embT")
            for g in range(4):
                embg = xe_g.tile([128, E], BF16, tag="embg")
                nc.gpsimd.indirect_dma_start(
                    embg[:],
                    None,
                    d["embW"][:],
                    IndirectOffsetOnAxis(ap=offs_s[:, g : g + 1], axis=0),
                )
                for ec in range(EC):
                    nc.sync.dma_start_transpose(
                        embT[:, ec, g * 128 : (g + 1) * 128],
                        embg[:, ec * 128 : (ec + 1) * 128],
                    )
            We_s = xe_tmp.tile([128, EC, 4 * H], BF16, tag="We_s")
            bg_s = xe_tmp.tile([1, 4 * H], BF16, tag="bg_s")
            nc.sync.dma_start(bg_s[:], d["bg"][:])
            for kc in range(EC):
                nc.sync.dma_start(We_s[:, kc, :], d["WeT"][kc * 128 : (kc + 1) * 128, :])
            for mc in range(GC):
                px = xe_ps.tile([128, BT], F32, tag="px")
                for kc in range(EC):
                    nc.tensor.matmul(
                        px[:],
                        We_s[:, kc, mc * 128 : (mc + 1) * 128],
                        embT[:, kc, :BT],
                        start=(kc == 0),
                        stop=False,
                    )
                nc.tensor.matmul(
                    px[:],
                    bg_s[0:1, mc * 128 : (mc + 1) * 128],
                    ones_s[0:1, :BT],
                    start=False,
                    stop=True,
                )
                nc.vector.tensor_copy(xegT[:, mc, :], px[:])

        # ---------- init recurrent state ----------
        nc.vector.memset(hT[:], 0.0)
        nc.vector.memset(cT[:], 0.0)
        nc.vector.memset(hpT[:], 0.0)
        nc.vector.memset(Hall[:], 0.0)

        alpha_pool = ctx.enter_context(tc.tile_pool(name="alpha_out", bufs=3))
        fc_w = ctx.enter_context(tc.tile_pool(name="fc_w", bufs=3))
        fc_ps = ctx.enter_context(
            tc.tile_pool(name="fc_ps", bufs=2, space=bass.MemorySpace.PSUM)
        )
        fc_c = ctx.enter_context(tc.tile_pool(name="fc_c", bufs=1))
        if has_fcb:
            fcb_s = fc_c.tile([1, V], BF16, tag="fcb_s")
            nc.sync.dma_start(fcb_s[:], d["fcb"][:])
            mrow_s = fc_c.tile([1, S * TP], BF16, tag="mrow_s")
            nc.sync.dma_start(mrow_s[:], d["maskRow"][:])
        else:
            fcb_s = mrow_s = None
        fc_pools = (fc_w, fc_ps, fcb_s, mrow_s)

        # ---------- the sequential scan (fc interleaved by t-chunks) ----------
        for t in range(T):
            m = int(m_sched[t])
            if m == 0:
                continue

            # hproj^T[e, b] = sum_k (w2/2)[e,k] H[b,k]   (H = 2h; b2 folded into FP)
            if t > 0:
                for j in range(EC):
                    for kc in range(HC):
                        nc.tensor.matmul(
                            ps_h[:, j, :m],
                            w2T[:, kc, j * 128 : (j + 1) * 128],
                            hT[:, kc, :m],
                            start=(kc == 0),
                            stop=(kc == HC - 1),
                        )
                nc.vector.tensor_copy(hpT[:, :, :m], ps_h[:, :, :m])

            # attention pipeline, chunked by echunk j: DVE add -> ACT tanh -> PE e-dot
            for j in range(EC):
                nc.vector.tensor_tensor(
                    spre[:, j, :, :m],
                    FP[:, j, :, :m],
                    hpT[:, j, :m]
                    .rearrange("pa (o s) -> pa o s", o=1)
                    .broadcast_to((128, P, m)),
                    ALU.add,
                )
                nc.scalar.activation(
                    score2[:, j, :m, :],
                    spre[:, j, :, :m].rearrange("pa p s -> pa s p"),
                    AF.Tanh,
                )
                for b in range(m):
                    for pc, (p0, pn) in enumerate(((0, P0N), (P0N, P1N))):
                        nc.tensor.matmul(
                            ps_eT[:pn, pc, j, b : b + 1],
                            score2[:, j, b, p0 : p0 + pn],
                            v_s[:, j : j + 1],
                            start=True,
                            stop=True,
                        )

            # sum the four echunk partials while moving psum -> sbuf
            with nc.allow_low_precision(reason="e logits tolerate bf16"):
                nc.vector.reduce_sum(
                    eT_sb[0:P0N, 0, :m],
                    ps_eT[0:P0N, 0, :, :m].rearrange("pa j b -> pa b j"),
                    axis=mybir.AxisListType.X,
                )
                nc.vector.reduce_sum(
                    eT_sb[0:P1N, 1, :m],
                    ps_eT[0:P1N, 1, :, :m].rearrange("pa j b -> pa b j"),
                    axis=mybir.AxisListType.X,
                )
            nc.tensor.transpose(
                ps_tr[:m, 0:P0N], eT_sb[0:P0N, 0, :m], eye_s[0:P0N, 0:P0N]
            )
            nc.tensor.transpose(
                ps_tr[:m, P0N:P], eT_sb[0:P1N, 1, :m], eye_s[0:P1N, 0:P1N]
            )

            # softmax over p (rows = slots)
            nc.vector.reduce_max(
                mx[:m, :], ps_tr[:m, 0:P], axis=mybir.AxisListType.X, negate=True
            )
            nc.scalar.activation(
                exp_s[:m, :], ps_tr[:m, 0:P], AF.Exp,
                bias=mx[:m, :], scale=1.0, accum_out=ssum[:m, :],
            )
            nc.vector.reciprocal(rs[:m, :], ssum[:m, :])
            nc.vector.tensor_mul(rs2[:m, :], rs[:m, :], maskP_s[:m, t : t + 1])
            a_out = alpha_pool.tile([S, P], F32, tag="a_out")
            nc.vector.tensor_scalar_mul(a_out[:m, :], exp_s[:m, :], rs2[:m, :])
            nc.sync.dma_start(d["alphas"][:m, t, :], a_out[:m, :])
            a_bf = alpha_pool.tile([S, P], BF16, tag="a_bf")
            nc.vector.tensor_scalar_mul(a_bf[:m, :], exp_s[:m, :], rs[:m, :])

            # transpose alpha -> [p, slot]
            nc.tensor.transpose(
                ps_tr[:P0N, 224 : 224 + m], a_bf[:m, 0:P0N], eye_s[:m, :m]
            )
            nc.tensor.transpose(
                ps_tr[:P1N, 256 : 256 + m], a_bf[:m, P0N:P], eye_s[:m, :m]
            )
            nc.vector.tensor_copy(alphaT[0:P0N, 0, :m], ps_tr[0:P0N, 224 : 224 + m])
            nc.vector.tensor_copy(alphaT[0:P1N, 1, :m], ps_tr[0:P1N, 256 : 256 + m])

            # ctx^T[e, b] = sum_p images[b, p, e] * alpha[b, p]
            for b in range(m):
                for j in range(EC):
                    nc.tensor.matmul(
                        ps_ctx[:, j, b : b + 1],
                        imgs0[:, b, j * 128 : (j + 1) * 128],
                        alphaT[0:P0N, 0, b : b + 1],
                        start=True,
                        stop=False,
                    )
                    nc.tensor.matmul(
                        ps_ctx[:, j, b : b + 1],
                        imgs1[:, b, j * 128 : (j + 1) * 128],
                        alphaT[0:P1N, 1, b : b + 1],
                        start=False,
                        stop=True,
                    )
            nc.vector.tensor_copy(xcT[:, :, :m], ps_ctx[:, :, :m])

            # gates^T = Wc @ ctx + (Wh/2) @ H  (+ xeg slice), order i,f,o,g
            for mc in range(GC):
                for kc in range(EC):
                    nc.tensor.matmul(
                        ps_g[:, mc, :m],
                        Wc[:, kc, mc * 128 : (mc + 1) * 128],
                        xcT[:, kc, :m],
                        start=(kc == 0),
                        stop=False,
                    )
                for kc in range(HC):
                    nc.tensor.matmul(
                        ps_g[:, mc, :m],
                        Wh[:, kc, mc * 128 : (mc + 1) * 128],
                        hT[:, kc, :m],
                        start=False,
                        stop=(kc == HC - 1),
                    )
            nc.vector.tensor_tensor(
                gsum[:, :, :m], ps_g[:, :, :m], xegT[:, :, t * S : t * S + m], ALU.add
            )

            # LSTM pointwise, all-tanh form (C = 2c, H = 2h):
            nc.scalar.activation(sig[:, :, :m], gsum[:, 0:12, :m], AF.Tanh, scale=0.5)
            nc.scalar.activation(tg[:, :, :m], gsum[:, 12:16, :m], AF.Tanh)
            nc.vector.scalar_tensor_tensor(
                u1[:, :, :m], sig[:, 0:4, :m], 1.0, tg[:, :, :m], ALU.add, ALU.mult
            )
            nc.vector.scalar_tensor_tensor(
                u2[:, :, :m], sig[:, 4:8, :m], 1.0, cT[:, :, :m], ALU.add, ALU.mult
            )
            nc.vector.scalar_tensor_tensor(
                cnew[:, :, :m], u2[:, :, :m], 0.5, u1[:, :, :m], ALU.mult, ALU.add
            )
            nc.vector.copy_predicated(
                cT[:, :, :m], mask4a[:, t, :, :m], cnew[:, :, :m]
            )
            nc.scalar.activation(tcs[:, :, :m], cT[:, :, :m], AF.Tanh, scale=0.5)
            nc.vector.scalar_tensor_tensor(
                hnew[:, :, :m], sig[:, 8:12, :m], 1.0, tcs[:, :, :m], ALU.add, ALU.mult
            )
            nc.vector.copy_predicated(
                hT[:, :, :m], mask4a[:, t, :, :m], hnew[:, :, :m]
            )
            nc.vector.tensor_mul(
                Hall[:, :, t, :m], hT[:, :, :m], maskb4a[:, t, :, :m]
            )

            # interleave fc for finished t-chunks (8 steps each) into the scan
            if t in (7, 15, 23):
                _fc_chunk(nc, d, fc_pools, Hall, t // 8, has_fcb)

        # ---------- final fc t-chunk (t 24..31) ----------
        _fc_chunk(nc, d, fc_pools, Hall, 3, has_fcb)


def _prep(inputs):
    images = np.asarray(inputs["images"], _f32)          # [B, P, E]
    captions = np.asarray(inputs["captions"])            # [B, L] int
    lengths = np.asarray(inputs["lengths"])              # [B] int
    embed_W = np.asarray(inputs["embed_W"], _f32)        # [V, E]
    w1 = np.asarray(inputs["w1"], _f32)                  # [E, E]
    b1 = np.asarray(inputs["b1"], _f32)
    w2 = np.asarray(inputs["w2"], _f32)                  # [E, H]
    b2 = np.asarray(inputs["b2"], _f32)
    v = np.asarray(inputs["v"], _f32)                    # [1, E]
    Wih = np.asarray(inputs["Wih"], _f32)                # [4H, 2E]
    bih = np.asarray(inputs["bih"], _f32)
    Whh = np.asarray(inputs["Whh"], _f32)                # [4H, H]
    bhh = np.asarray(inputs["bhh"], _f32)
    fcW = np.asarray(inputs["fcW"], _f32)                # [V, H]
    fcb = np.asarray(inputs["fcb"], _f32)

    dec = np.clip(lengths.astype(np.int64) - 1, 0, T)    # [B]

    order = np.argsort(-dec, kind="stable")
    core_slots = [
        [int(order[s * NCORES + j]) for s in range(S)] for j in range(NCORES)
    ]
    N_t = np.array([(dec > t).sum() for t in range(T)], np.int64)
    m_sched = [int(-(-n // NCORES)) for n in N_t]

    # gate-row permutation: torch order i,f,g,o -> i,f,o,g
    perm = np.concatenate(
        [np.arange(0, H), np.arange(H, 2 * H), np.arange(3 * H, 4 * H),
         np.arange(2 * H, 3 * H)]
    )
    WihE = Wih[:, :E][perm]
    WihC = Wih[:, E:][perm]
    Whh_p = Whh[perm]
    bg = (bih + bhh)[perm]

    def bf(x):
        return np.ascontiguousarray(x.astype(_bf16))

    shared = {
        "w1T": bf(w1.T),
        "w2T": bf(w2.T * 0.5),                 # h = H/2 folded in
        "b12": bf((b1 + b2)[None, :]),
        "vv": bf(v[0][:, None]),
        "WcT": bf(WihC.T),
        "WhT": bf(Whh_p.T * 0.5),
        "WeT": bf(WihE.T),
        "bg": bf(bg[None, :]),
        "embW": bf(embed_W),
        "fcWT": bf(fcW.T * 0.5),
        "fcb": bf(fcb[None, :]),
        "eye": np.eye(128, dtype=_bf16),
    }
    has_fcb = bool(np.any(fcb != 0.0))

    in_maps = []
    for j in range(NCORES):
        sl = core_slots[j]
        img_c = images[sl]                      # [S, P, E]
        dec_c = dec[sl]
        cap_c = np.asarray(captions)[sl]

        flat = np.zeros(BTP, np.uint32)
        for t in range(T):
            flat[t * S : t * S + S] = cap_c[:, t].astype(np.uint32)
        offs = flat.reshape(4, 128)

        tgrid = np.arange(T)[:, None]           # [T, 1]
        on = (tgrid < dec_c[None, :]).astype(_f32)   # [T, S]
        maskf = on.reshape(1, BT)
        maskP = np.ascontiguousarray(on.T)      # [S, T]
        maskRow = np.zeros((TP, S), _f32)
        maskRow[:T, :] = on
        maskRow = maskRow.reshape(1, TP * S).astype(_bf16)

        m = dict(shared)
        m["imT"] = bf(np.ascontiguousarray(img_c.transpose(2, 0, 1)))   # [E, S, P]
        m["imgs"] = bf(np.ascontiguousarray(img_c.transpose(1, 0, 2)))  # [P, S, E]
        m["offs"] = offs
        m["maskf"] = maskf
        m["maskP"] = maskP
        m["maskRow"] = maskRow
        in_maps.append(m)

    return in_maps, m_sched, has_fcb, core_slots


last_results = None  # BassKernelResults of the most recent kernel() call


def kernel(**inputs):
    global last_results
    in_maps, m_sched, has_fcb, core_slots = _prep(inputs)
    nc = _build_program(m_sched, has_fcb)
    res = bass_utils.run_bass_kernel_spmd(nc, in_maps, core_ids=list(range(NCORES)))
    last_results = res
    outs = res.results

    predictions = np.zeros((B, L, V), _f32)
    alphas = np.zeros((B, L, P), _f32)
    for j in range(NCORES):
        pr = np.asarray(outs[j]["preds"], _f32)    # [S, TP, V]
        al = np.asarray(outs[j]["alphas"], _f32)   # [S, TP, P]
        for s in range(S):
            b = core_slots[j][s]
            predictions[b] = pr[s]
            alphas[b] = al[s]

    return predictions, inputs["captions"], inputs["lengths"], alphas
